# revision 1
# baseline (speedup 1.0000x reference)
"""ConsistencyLoss kernel v2 for 8 Trainium2 NeuronCores.

Math (per reference):
  For view1: sim = cos_sim_pairwise(y1, z2) [B,N,N]; mask from grid distances;
  loss_v = sum(sim*mask)/sum(mask); out = -(loss_1 + loss_2), N = 28*28 = 784.

v2 strategy (data-parallel over batch, 8 batches/core):
  - Both cosine norms folded into the features on host; features shipped as
    float16 (halves HBM traffic; fp16 matmul runs 1 cycle/row on PE).
    Measured host-side fp16 end-to-end rel err ~2.8e-4 (vs 2e-2 gate).
  - Features packed per view-pair ([128, 2tensor, 2cc, N]) and aux tables
    packed per batch -> 3 DMAs per batch + 1 upfront: HWDGE fixed cost
    (625ns/DMA) stays off the critical path.
  - n tiled as 7 x 112 partitions (112 = 4 image rows exactly), so each
    tile's mask window is only WW (~6) image rows of m (WW*28 columns) and
    the per-partition x-distance table is k-independent (j = p mod 28).
  - Window offsets: one upfront DMA for all batches; ONE TensorLoad per
    batch fills 7 PE registers (v1 did 56 loads = 51us of PE.SEQ).
  - PSUM: two k-tiles' numerators per bank ([112, 2, WW*28] fp32), so the
    mask+accumulate pass (scalar_tensor_tensor: (d2 <= t^2) * num with
    per-partition accum) runs once per k-pair, amortizing the 125ns PSUM
    access penalty. stt ops are statically load-balanced between DVE and
    GpSimd (Pool); d2 assembly stays on DVE.
  - Per-(batch,view,group) partial sums land in per-engine accumulator
    tiles; final reduction on host (the all-reduce of the sharding hint),
    along with exact fp32 mask counts for the denominators.
"""

import sys

sys.path.insert(0, "/opt/trn_rl_repo")

import numpy as np

import concourse.bass as bass
import concourse.mybir as mybir
import concourse.tile as tile
from concourse import bacc
from concourse.bass import broadcast_tensor_aps
from concourse.bass_utils import run_bass_kernel_spmd

B, C, H, W = 64, 256, 28, 28
N = H * W  # 784
NCORES = 8
BPC = B // NCORES  # batches per core
G = 4  # image rows of n per tile
P = G * 28  # 112 partitions per tile
NT = N // P  # 7 tiles, exact
THR = 0.7

F32 = mybir.dt.float32
F16 = mybir.dt.float16
I32 = mybir.dt.int32
ALU = mybir.AluOpType
ENG = mybir.EngineType

_COMPILED = {}


def _kgroups(WW):
    # pack as many k-tiles per PSUM bank (512 fp32) as fit: amortizes the
    # 125ns PSUM access cost of each mask+accumulate op and frees banks
    if 3 * WW * 28 <= 512:
        return [(0, 1, 2), (3, 4, 5), (6,)]
    if 2 * WW * 28 <= 512:
        return [(0, 1), (2, 3), (4, 5), (6,)]
    return [(k,) for k in range(NT)]


def _build_nc(WW=6, reps=None, woff_static=None, skip_compute=False):
    # reps: timing instrumentation only — wraps the whole computation in a
    # hardware loop so one dispatch executes it `reps` times (results are
    # identical each iteration; used to measure HW time via wall-clock slope).
    # woff_static: tuple of BPC*NT window offsets baked in at compile time
    # (kills the per-matmul dynamic-AP ISA ops and the register loads on
    # PE.SEQ); None falls back to runtime register offsets.
    WCOL = WW * 28
    AUXW = NT * WW + 30  # dyw | dxp | thr packed per partition
    kgroups = _kgroups(WW)
    NG = len(kgroups)

    nc = bacc.Bacc("TRN2", debug=False, num_devices=NCORES)

    ins = {
        # f1 = view1 pair (y1_hat, z2_hat); f2 = view2 pair (y2_hat, z1_hat)
        # cc (contraction half) is the outer dim so each DMA is half a pair.
        "f1": nc.dram_tensor("f1", [BPC, 2, 128, 2, N], F16, kind="ExternalInput"),
        "f2": nc.dram_tensor("f2", [BPC, 2, 128, 2, N], F16, kind="ExternalInput"),
        "aux": nc.dram_tensor("aux", [BPC, P, AUXW], F32, kind="ExternalInput"),
        "woff": nc.dram_tensor("woff", [1, BPC * NT], I32, kind="ExternalInput"),
    }
    out = nc.dram_tensor("out", [P, BPC * 2 * NG], F32, kind="ExternalOutput")

    with tile.TileContext(nc) as tc:
        with (
            tc.tile_pool(name="feat", bufs=3) as feat_pool,
            tc.tile_pool(name="aux", bufs=3) as aux_pool,
            tc.tile_pool(name="d2", bufs=2) as d2_pool,
            tc.tile_pool(name="acc", bufs=1) as acc_pool,
            tc.tile_pool(name="psum", bufs=1, space="PSUM") as psum_pool,
        ):
            # every (b, v, g) column is written exactly once by a DVE stt
            ms_all = acc_pool.tile([P, BPC, 2, NG], F32, name="ms_all", tag="ms")

            woff_t = acc_pool.tile([1, BPC * NT], I32, name="woff_t", tag="woff")

            import contextlib

            loop_ctx = tc.For_i(0, reps) if reps else contextlib.nullcontext()
            with loop_ctx:
              all_wvs = None
              for b in range(BPC):
                # first feature DMA goes out first so the tiny aux/woff
                # transfers' HWDGE setup hides under its transfer time
                feats = {}
                order = [("f1", 0), ("f1", 1), ("f2", 0), ("f2", 1)]
                for idx, (nm, cc) in enumerate(order):
                    t = feat_pool.tile([128, 2, N], F16, tag=f"{nm}c{cc}")
                    nc.sync.dma_start(t[:, :, :], ins[nm][b, cc])
                    feats[(nm, cc)] = t
                    if idx == 0:
                        if b == 0 and woff_static is None:
                            nc.sync.dma_start(woff_t[:, :], ins["woff"][:, :])
                        aux_t = aux_pool.tile([P, AUXW], F32, tag="aux")
                        nc.sync.dma_start(aux_t[:, :], ins["aux"][b])
                dyw_v = aux_t[:, 0 : NT * WW].rearrange("p (k w) -> p k w", k=NT)
                dxp_v = aux_t[:, NT * WW : NT * WW + 28]
                thr_v = aux_t[:, NT * WW + 28 : NT * WW + 30]

                if skip_compute:
                    continue
                if woff_static is not None:
                    wvs = [int(woff_static[b * NT + k]) for k in range(NT)]
                else:
                    if b % 2 == 0:
                        # one TensorLoad per two batches (register-file bound)
                        _, all_wvs = nc.values_load_multi_w_load_instructions(
                            woff_t[0:1, b * NT : (b + 2) * NT],
                            engines=(ENG.PE,),
                            min_val=0,
                            max_val=(28 - WW) * 28,
                            skip_runtime_bounds_check=True,
                        )
                    wvs = all_wvs[(b % 2) * NT : (b % 2 + 1) * NT]

                for g, ks in enumerate(kgroups):
                    L = len(ks)
                    d2 = d2_pool.tile([P, L, WCOL], F32, tag=f"d2_{g}")
                    i0, i1 = broadcast_tensor_aps(
                        dyw_v[:, ks[0] : ks[0] + L, :, None],
                        dxp_v[:, None, None, :],
                    )
                    # d2 assembly on GpSimd (SBUF-only op; Pool cannot touch
                    # PSUM, so DVE keeps every PSUM-reading stt below)
                    nc.gpsimd.tensor_tensor(
                        d2[:, :, :].rearrange("q l (a c) -> q l a c", a=WW),
                        i0,
                        i1,
                        ALU.add,
                    )
                    for v, fnm in enumerate(("f1", "f2")):
                        num = psum_pool.tile([P, L, WCOL], F32, tag=f"num{g}_{v}")
                        for li, k in enumerate(ks):
                            for cc in (0, 1):
                                ft = feats[(fnm, cc)]
                                if isinstance(wvs[k], int):
                                    mv = ft[:, 1, wvs[k] : wvs[k] + WCOL]
                                else:
                                    mv = ft[:, 1, bass.ds(wvs[k], WCOL)]
                                nc.tensor.matmul(
                                    num[:, li, :],
                                    ft[:, 0, k * P : (k + 1) * P],
                                    mv,
                                    start=(cc == 0),
                                    stop=(cc == 1),
                                )
                        nc.vector.scalar_tensor_tensor(
                            out=num[:, :, :],
                            in0=d2[:, :, :],
                            scalar=thr_v[:, v : v + 1],
                            in1=num[:, :, :],
                            op0=ALU.is_le,
                            op1=ALU.mult,
                            accum_out=ms_all[:, b, v, g : g + 1],
                        )

            nc.sync.dma_start(out[:, :], ms_all[:, :, :, :])

    nc.compile()
    return nc


def _get_nc(WW, woff_static=None):
    # Prefer baking the data-dependent window offsets into the program
    # (removes all dynamic addressing from PE.SEQ). If a session sees many
    # distinct offset tables (unexpected), stop recompiling and fall back to
    # the register-offset variant.
    if woff_static is not None and sum(k[1] is not None for k in _COMPILED) < 4:
        key = (WW, woff_static)
    else:
        key = (WW, None)
    if key not in _COMPILED:
        _COMPILED[key] = _build_nc(WW, woff_static=key[1])
    return _COMPILED[key]


def _prep_host(y1, y2, z1, z2, view1_grid, view2_grid):
    """Host-side prep: separable distance tables, norms, counts, shards."""
    y1f = y1.reshape(B, C, N)
    y2f = y2.reshape(B, C, N)
    z1f = z1.reshape(B, C, N)
    z2f = z2.reshape(B, C, N)

    # --- separable grid tables ------------------------------------------
    g1y = view1_grid[:, 0, :, 0]  # [B, 28]
    g1x = view1_grid[:, 1, 0, :]
    g2y = view2_grid[:, 0, :, 0]
    g2x = view2_grid[:, 1, 0, :]
    if not (
        np.array_equal(view1_grid[:, 0], np.broadcast_to(g1y[:, :, None], (B, H, W)))
        and np.array_equal(view1_grid[:, 1], np.broadcast_to(g1x[:, None, :], (B, H, W)))
        and np.array_equal(view2_grid[:, 0], np.broadcast_to(g2y[:, :, None], (B, H, W)))
        and np.array_equal(view2_grid[:, 1], np.broadcast_to(g2x[:, None, :], (B, H, W)))
    ):
        raise RuntimeError("grids are not separable; unsupported input")

    dy = g1y[:, :, None] - g2y[:, None, :]  # fp32 [B,28,28]
    dx = g1x[:, :, None] - g2x[:, None, :]
    dy2 = dy * dy
    dx2 = dx * dx

    v1bin = np.linalg.norm(view1_grid[..., 1, 1] - view1_grid[..., 0, 0], axis=-1)
    v2bin = np.linalg.norm(view2_grid[..., 1, 1] - view2_grid[..., 0, 0], axis=-1)
    t2 = np.empty((B, 2), np.float32)
    t2[:, 0] = ((THR * v1bin.astype(np.float64)) ** 2).astype(np.float32)
    t2[:, 1] = ((THR * v2bin.astype(np.float64)) ** 2).astype(np.float32)

    # --- per-(batch, tile) windows of valid i' --------------------------
    tmax2 = np.maximum(t2[:, 0], t2[:, 1]).astype(np.float64) * (1 + 1e-6)  # [B]
    first = np.zeros((B, NT), np.int64)
    width = np.zeros((B, NT), np.int64)
    anyv = np.zeros((B, NT), bool)
    for k in range(NT):
        sub_min = dy2[:, G * k : G * k + G, :].min(axis=1)  # [B, 28]
        valid = sub_min <= tmax2[:, None]  # [B, 28]
        anyv[:, k] = valid.any(axis=1)
        first[:, k] = np.argmax(valid, axis=1)
        last = 27 - np.argmax(valid[:, ::-1], axis=1)
        width[:, k] = np.where(anyv[:, k], last - first[:, k] + 1, 1)
    WW = int(max(width.max(), 4))
    if WW > 18:
        raise RuntimeError(f"mask window {WW} rows; unsupported input")

    w0 = np.minimum(np.where(anyv, first, 0), 28 - WW).astype(np.int64)  # [B, NT]

    # NOTE: window offsets differ per batch and the program is SPMD (one
    # NEFF on all 8 cores), so the offsets stay runtime register values.
    # (A host-side row-roll scheme that equalizes offsets across cores was
    # explored — it needs the window widened by the per-slot profile spread,
    # which costs more PE/DVE time than the saved register ops.)
    iidx = (np.arange(P) // 28).astype(np.int64)  # [112] in 0..3
    dyw = np.zeros((B, P, NT, WW), np.float32)
    for k in range(NT):
        rows = G * k + iidx  # [112] y-side image rows
        cols = w0[:, k][:, None] + np.arange(WW)[None, :]  # [B, WW]
        dyw[:, :, k] = dy2[
            np.arange(B)[:, None, None], rows[None, :, None], cols[:, None, :]
        ]
    woff = (w0 * 28).astype(np.int32)  # [B, NT]
    woff_static = None

    AUXW = NT * WW + 30
    aux = np.zeros((B, P, AUXW), np.float32)
    aux[:, :, 0 : NT * WW] = dyw.reshape(B, P, NT * WW)
    aux[:, :, NT * WW : NT * WW + 28] = np.tile(dx2, (1, G, 1))  # j = p mod 28
    aux[:, :, NT * WW + 28 : NT * WW + 30] = t2[:, None, :]

    # --- mask counts (bit-identical fp32 add + compare as device) -------
    counts = np.zeros(2, np.int64)
    for b in range(B):
        d2b = dy2[b][:, None, :, None] + dx2[b][None, :, None, :]  # fp32
        counts[0] += int((d2b <= t2[b, 0]).sum())
        counts[1] += int((d2b <= t2[b, 1]).sum())

    # --- normalized features in fp16, packed per view pair --------------
    def normed(a):
        n = np.sqrt(np.einsum("bcn,bcn->bn", a, a, dtype=np.float32))
        return a * (1.0 / np.maximum(n, np.float32(1e-7)))[:, None, :]

    def pack_pair(ya, zb):
        # two [B, C, N] fp32 -> [B, 2cc, 128, 2tensor, N] fp16
        f = np.empty((B, 2, 128, 2, N), np.float16)
        f[:, :, :, 0] = normed(ya).reshape(B, 2, 128, N)
        f[:, :, :, 1] = normed(zb).reshape(B, 2, 128, N)
        return f

    f1 = pack_pair(y1f, z2f)
    f2 = pack_pair(y2f, z1f)

    in_maps = []
    for c in range(NCORES):
        s = slice(c * BPC, (c + 1) * BPC)
        in_maps.append(
            {
                "f1": f1[s],
                "f2": f2[s],
                "aux": aux[s],
                "woff": np.ascontiguousarray(woff[s].reshape(1, BPC * NT)),
            }
        )
    return in_maps, counts, WW, woff_static


def kernel(y1, y2, z1, z2, view1_grid, view2_grid):
    y1 = np.asarray(y1, np.float32)
    y2 = np.asarray(y2, np.float32)
    z1 = np.asarray(z1, np.float32)
    z2 = np.asarray(z2, np.float32)
    view1_grid = np.asarray(view1_grid, np.float32)
    view2_grid = np.asarray(view2_grid, np.float32)

    in_maps, counts, WW, woff_static = _prep_host(
        y1, y2, z1, z2, view1_grid, view2_grid
    )
    # Offsets are core-uniform (host rolled the z planes), so they compile
    # in statically; _get_nc falls back to the register variant if a session
    # sees many distinct offset tables.
    nc = _get_nc(WW, woff_static=woff_static)
    res = run_bass_kernel_spmd(nc, in_maps, core_ids=list(range(NCORES)))
    s = np.zeros(2, np.float64)
    for i in range(NCORES):
        o = res.results[i]["out"].astype(np.float64)  # [P, BPC*2*NG]
        ng = o.shape[1] // (BPC * 2)
        o = o.reshape(P, BPC, 2, ng)
        s += o.sum(axis=(0, 1, 3))
    loss = -(
        np.float32(s[0]) / np.float32(counts[0])
        + np.float32(s[1]) / np.float32(counts[1])
    )
    return np.array(loss, dtype=np.float32)



# revision 3
# speedup vs baseline: 1.2900x; 1.2900x over previous
"""ConsistencyLoss kernel v3 for 8 Trainium2 NeuronCores.

Math (per reference):
  For view1: sim = cos_sim_pairwise(y1, z2) [B,N,N]; mask from grid distances;
  loss_v = sum(sim*mask)/sum(mask); out = -(loss_1 + loss_2), N = 28*28 = 784.

v3 strategy (data-parallel over batch, 8 batches/core), changes vs v2:
  - Features in fp8 e4m3 (ml_dtypes.float8_e4m3 == mybir float8e4). Measured
    end-to-end rel err 5.1e-3 on the harness inputs (gate 2e-2). Halves HBM
    traffic vs fp16 AND enables the PE DoubleRow perf mode.
  - Moving (z-side) windows gathered on the HOST into a [128,2,2,NT,WCOL]
    blob: every matmul AP is compile-time static. This kills the dynamic-AP
    ISA patches (21.5us of PE.SEQ in v2) and the TensorLoads, at the cost of
    shipping z windows 1.5x (4704B vs 3136B per partition per batch).
  - DoubleRow fp8 matmul: lhsT [128,2cc,112], rhs [128,2cc,168] -> one
    matmul per (batch, view, k-tile) covers the full 256-channel contraction
    at 0.5 cycles/row. 112 matmuls + 112 ldweights total (v2: 224 + 224).
  - One fused feature DMA per batch ([128, 7840B] blob) + one upfront aux
    DMA + one output DMA = 10 DMAs total (v2: 42). DMA engines move
    8*128*7840B = 8.03MB/core ~= 22.3us at the modeled 360GB/s.
  - d2 assembly on Pool (SBUF-only), mask+multiply+accumulate stt on DVE
    reading num from PSUM, unchanged from v2.
  - Per-(batch,view,group) partial sums land in an accumulator tile; final
    reduction on host (the all-reduce of the sharding hint), along with
    exact fp32 mask counts for the denominators.
"""

import sys

sys.path.insert(0, "/opt/trn_rl_repo")

import ml_dtypes
import numpy as np

import concourse.bass as bass
import concourse.mybir as mybir
import concourse.tile as tile
from concourse import bacc
from concourse.bass import broadcast_tensor_aps
from concourse.bass_utils import run_bass_kernel_spmd

B, C, H, W = 64, 256, 28, 28
N = H * W  # 784
NCORES = 8
BPC = B // NCORES  # batches per core
G = 4  # image rows of n per tile
P = G * 28  # 112 partitions per tile
NT = N // P  # 7 tiles, exact
THR = 0.7

F32 = mybir.dt.float32
F8 = mybir.dt.float8e4
FP8_NP = ml_dtypes.float8_e4m3
ALU = mybir.AluOpType
ENG = mybir.EngineType
DOUBLE_ROW = mybir.MatmulPerfMode.DoubleRow

_COMPILED = {}


def _kgroups(WW):
    # pack as many k-tiles per PSUM bank (512 fp32) as fit: amortizes the
    # PSUM access cost of each mask+accumulate op and frees banks
    if 3 * WW * 28 <= 512:
        return [(0, 1, 2), (3, 4, 5), (6,)]
    if 2 * WW * 28 <= 512:
        return [(0, 1), (2, 3), (4, 5), (6,)]
    return [(k,) for k in range(NT)]


def _build_nc(WW=6):
    WCOL = WW * 28
    AUXW = NT * WW + 28 + 2  # dyw | dxp | thr packed per partition
    YB = 2 * 2 * N  # y blob bytes/partition (v, cc, n) fp8
    ZB = 2 * 2 * NT * WCOL  # z window blob bytes/partition (v, cc, k, w)
    FB = YB + ZB
    kgroups = _kgroups(WW)
    NG = len(kgroups)

    nc = bacc.Bacc("TRN2", debug=False, num_devices=NCORES)

    ins = {
        # per-batch fused feature blob: y-pack [128,2v,2cc,N] then z-window
        # pack [128,2v,2cc,NT,WCOL], all fp8 -> a single DMA per batch
        "feat": nc.dram_tensor("feat", [BPC, 128, FB], F8, kind="ExternalInput"),
        # all batches' aux tables in one upfront DMA
        "aux": nc.dram_tensor("aux", [P, BPC * AUXW], F32, kind="ExternalInput"),
    }
    out = nc.dram_tensor("out", [P, BPC * 2 * NG], F32, kind="ExternalOutput")

    with tile.TileContext(nc) as tc:
        with (
            tc.tile_pool(name="feat", bufs=3) as feat_pool,
            tc.tile_pool(name="aux", bufs=1) as aux_pool,
            tc.tile_pool(name="d2", bufs=2) as d2_pool,
            tc.tile_pool(name="acc", bufs=1) as acc_pool,
            tc.tile_pool(name="psum", bufs=1, space="PSUM") as psum_pool,
        ):
            # every (b, v, g) column is written exactly once by a DVE stt
            ms_all = acc_pool.tile([P, BPC, 2, NG], F32, name="ms_all", tag="ms")

            aux_t = aux_pool.tile([P, BPC * AUXW], F32, tag="aux")
            nc.sync.dma_start(aux_t[:, :], ins["aux"][:, :])

            for b in range(BPC):
                ft = feat_pool.tile([128, FB], F8, tag="feat")
                nc.sync.dma_start(ft[:, :], ins["feat"][b])
                yv = ft[:, 0:YB].rearrange("p (v c n) -> p v c n", v=2, c=2)
                zv = ft[:, YB:FB].rearrange("p (v c k w) -> p v c k w", v=2, c=2, k=NT)

                a0 = b * AUXW
                dyw_v = aux_t[:, a0 : a0 + NT * WW].rearrange(
                    "p (k w) -> p k w", k=NT
                )
                dxp_v = aux_t[:, a0 + NT * WW : a0 + NT * WW + 28]
                thr_v = aux_t[:, a0 + NT * WW + 28 : a0 + NT * WW + 30]

                for g, ks in enumerate(kgroups):
                    L = len(ks)
                    d2 = d2_pool.tile([P, L, WCOL], F32, tag=f"d2_{g}")
                    i0, i1 = broadcast_tensor_aps(
                        dyw_v[:, ks[0] : ks[0] + L, :, None],
                        dxp_v[:, None, None, :],
                    )
                    # d2 assembly on GpSimd (SBUF-only op; Pool cannot touch
                    # PSUM, so DVE keeps every PSUM-reading stt below)
                    nc.gpsimd.tensor_tensor(
                        d2[:, :, :].rearrange("q l (a c) -> q l a c", a=WW),
                        i0,
                        i1,
                        ALU.add,
                    )
                    for v in range(2):
                        num = psum_pool.tile([P, L, WCOL], F32, tag=f"num{g}_{v}")
                        for li, k in enumerate(ks):
                            # full 256-channel contraction in one DoubleRow
                            # fp8 matmul: lhsT [128,2,112], rhs [128,2,WCOL]
                            nc.tensor.matmul(
                                num[:, li, :],
                                yv[:, v, :, k * P : (k + 1) * P],
                                zv[:, v, :, k, :],
                                start=True,
                                stop=True,
                                perf_mode=DOUBLE_ROW,
                            )
                        nc.vector.scalar_tensor_tensor(
                            out=num[:, :, :],
                            in0=d2[:, :, :],
                            scalar=thr_v[:, v : v + 1],
                            in1=num[:, :, :],
                            op0=ALU.is_le,
                            op1=ALU.mult,
                            accum_out=ms_all[:, b, v, g : g + 1],
                        )

            nc.sync.dma_start(out[:, :], ms_all[:, :, :, :])

    nc.compile()
    return nc


def _get_nc(WW):
    if WW not in _COMPILED:
        _COMPILED[WW] = _build_nc(WW)
    return _COMPILED[WW]


def _prep_host(y1, y2, z1, z2, view1_grid, view2_grid):
    """Host-side prep: separable distance tables, norms, counts, shards."""
    y1f = y1.reshape(B, C, N)
    y2f = y2.reshape(B, C, N)
    z1f = z1.reshape(B, C, N)
    z2f = z2.reshape(B, C, N)

    # --- separable grid tables ------------------------------------------
    g1y = view1_grid[:, 0, :, 0]  # [B, 28]
    g1x = view1_grid[:, 1, 0, :]
    g2y = view2_grid[:, 0, :, 0]
    g2x = view2_grid[:, 1, 0, :]
    if not (
        np.array_equal(view1_grid[:, 0], np.broadcast_to(g1y[:, :, None], (B, H, W)))
        and np.array_equal(view1_grid[:, 1], np.broadcast_to(g1x[:, None, :], (B, H, W)))
        and np.array_equal(view2_grid[:, 0], np.broadcast_to(g2y[:, :, None], (B, H, W)))
        and np.array_equal(view2_grid[:, 1], np.broadcast_to(g2x[:, None, :], (B, H, W)))
    ):
        raise RuntimeError("grids are not separable; unsupported input")

    dy = g1y[:, :, None] - g2y[:, None, :]  # fp32 [B,28,28]
    dx = g1x[:, :, None] - g2x[:, None, :]
    dy2 = dy * dy
    dx2 = dx * dx

    v1bin = np.linalg.norm(view1_grid[..., 1, 1] - view1_grid[..., 0, 0], axis=-1)
    v2bin = np.linalg.norm(view2_grid[..., 1, 1] - view2_grid[..., 0, 0], axis=-1)
    t2 = np.empty((B, 2), np.float32)
    t2[:, 0] = ((THR * v1bin.astype(np.float64)) ** 2).astype(np.float32)
    t2[:, 1] = ((THR * v2bin.astype(np.float64)) ** 2).astype(np.float32)

    # --- per-(batch, tile) windows of valid i' --------------------------
    tmax2 = np.maximum(t2[:, 0], t2[:, 1]).astype(np.float64) * (1 + 1e-6)  # [B]
    first = np.zeros((B, NT), np.int64)
    width = np.zeros((B, NT), np.int64)
    anyv = np.zeros((B, NT), bool)
    for k in range(NT):
        sub_min = dy2[:, G * k : G * k + G, :].min(axis=1)  # [B, 28]
        valid = sub_min <= tmax2[:, None]  # [B, 28]
        anyv[:, k] = valid.any(axis=1)
        first[:, k] = np.argmax(valid, axis=1)
        last = 27 - np.argmax(valid[:, ::-1], axis=1)
        width[:, k] = np.where(anyv[:, k], last - first[:, k] + 1, 1)
    WW = int(max(width.max(), 4))
    if WW > 18:
        raise RuntimeError(f"mask window {WW} rows; unsupported input")
    WCOL = WW * 28

    w0 = np.minimum(np.where(anyv, first, 0), 28 - WW).astype(np.int64)  # [B, NT]

    iidx = (np.arange(P) // 28).astype(np.int64)  # [112] in 0..3
    dyw = np.zeros((B, P, NT, WW), np.float32)
    for k in range(NT):
        rows = G * k + iidx  # [112] y-side image rows
        cols = w0[:, k][:, None] + np.arange(WW)[None, :]  # [B, WW]
        dyw[:, :, k] = dy2[
            np.arange(B)[:, None, None], rows[None, :, None], cols[:, None, :]
        ]

    AUXW = NT * WW + 28 + 2
    aux = np.zeros((B, P, AUXW), np.float32)
    aux[:, :, 0 : NT * WW] = dyw.reshape(B, P, NT * WW)
    aux[:, :, NT * WW : NT * WW + 28] = np.tile(dx2, (1, G, 1))  # j = p mod 28
    aux[:, :, NT * WW + 28 : NT * WW + 30] = t2[:, None, :]

    # --- mask counts (bit-identical fp32 add + compare as device) -------
    counts = np.zeros(2, np.int64)
    for b in range(B):
        d2b = dy2[b][:, None, :, None] + dx2[b][None, :, None, :]  # fp32
        counts[0] += int((d2b <= t2[b, 0]).sum())
        counts[1] += int((d2b <= t2[b, 1]).sum())

    # --- normalized features in fp8, fused per-batch blob ---------------
    def normed8(a):
        n = np.sqrt(np.einsum("bcn,bcn->bn", a, a, dtype=np.float32))
        h = a * (1.0 / np.maximum(n, np.float32(1e-7)))[:, None, :]
        return h.reshape(B, 2, 128, N).astype(FP8_NP)  # [B, cc, part, n]

    y1h, y2h, z1h, z2h = normed8(y1f), normed8(y2f), normed8(z1f), normed8(z2f)

    # y-pack [B, part, v, cc, n]
    ypack = np.empty((B, 128, 2, 2, N), FP8_NP)
    ypack[:, :, 0] = y1h.transpose(0, 2, 1, 3)
    ypack[:, :, 1] = y2h.transpose(0, 2, 1, 3)

    # z window pack [B, part, v, cc, k, w]; v=0 pairs with z2, v=1 with z1
    zpack = np.empty((B, 128, 2, 2, NT, WCOL), FP8_NP)
    bi = np.arange(B)[:, None, None, None]  # [B,1,1,1]
    pi = np.arange(128)[None, :, None, None]  # [1,128,1,1]
    wi = w0 * 28  # [B, NT] window start columns
    cols = wi[:, :, None] + np.arange(WCOL)[None, None, :]  # [B, NT, WCOL]
    ci = cols[:, None, :, :]  # [B,1,NT,WCOL]
    for cc in range(2):
        # gather [B, part, NT, WCOL] from [B, part, N]
        zpack[:, :, 0, cc] = z2h[:, cc][bi, pi, ci]
        zpack[:, :, 1, cc] = z1h[:, cc][bi, pi, ci]

    FBB = 2 * 2 * N + 2 * 2 * NT * WCOL
    feat = np.concatenate(
        [ypack.reshape(B, 128, 2 * 2 * N), zpack.reshape(B, 128, 2 * 2 * NT * WCOL)],
        axis=2,
    )
    assert feat.shape == (B, 128, FBB)

    in_maps = []
    for c in range(NCORES):
        s = slice(c * BPC, (c + 1) * BPC)
        in_maps.append(
            {
                "feat": feat[s],
                "aux": np.ascontiguousarray(
                    aux[s].transpose(1, 0, 2).reshape(P, BPC * AUXW)
                ),
            }
        )
    return in_maps, counts, WW


def kernel(y1, y2, z1, z2, view1_grid, view2_grid):
    y1 = np.asarray(y1, np.float32)
    y2 = np.asarray(y2, np.float32)
    z1 = np.asarray(z1, np.float32)
    z2 = np.asarray(z2, np.float32)
    view1_grid = np.asarray(view1_grid, np.float32)
    view2_grid = np.asarray(view2_grid, np.float32)

    in_maps, counts, WW = _prep_host(y1, y2, z1, z2, view1_grid, view2_grid)
    nc = _get_nc(WW)
    res = run_bass_kernel_spmd(nc, in_maps, core_ids=list(range(NCORES)))
    s = np.zeros(2, np.float64)
    for i in range(NCORES):
        o = res.results[i]["out"].astype(np.float64)  # [P, BPC*2*NG]
        ng = o.shape[1] // (BPC * 2)
        o = o.reshape(P, BPC, 2, ng)
        s += o.sum(axis=(0, 1, 3))
    loss = -(
        np.float32(s[0]) / np.float32(counts[0])
        + np.float32(s[1]) / np.float32(counts[1])
    )
    return np.array(loss, dtype=np.float32)


# revision 9
# speedup vs baseline: 1.2997x; 1.0075x over previous
"""ConsistencyLoss kernel v3 for 8 Trainium2 NeuronCores.

Math (per reference):
  For view1: sim = cos_sim_pairwise(y1, z2) [B,N,N]; mask from grid distances;
  loss_v = sum(sim*mask)/sum(mask); out = -(loss_1 + loss_2), N = 28*28 = 784.

v3 strategy (data-parallel over batch, 8 batches/core), changes vs v2:
  - Features in fp8 e4m3 (ml_dtypes.float8_e4m3 == mybir float8e4). Measured
    end-to-end rel err 5.1e-3 on the harness inputs (gate 2e-2). Halves HBM
    traffic vs fp16 AND enables the PE DoubleRow perf mode.
  - Moving (z-side) windows gathered on the HOST into a [128,2,2,NT,WCOL]
    blob: every matmul AP is compile-time static. This kills the dynamic-AP
    ISA patches (21.5us of PE.SEQ in v2) and the TensorLoads, at the cost of
    shipping z windows 1.5x (4704B vs 3136B per partition per batch).
  - DoubleRow fp8 matmul: lhsT [128,2cc,112], rhs [128,2cc,168] -> one
    matmul per (batch, view, k-tile) covers the full 256-channel contraction
    at 0.5 cycles/row. 112 matmuls + 112 ldweights total (v2: 224 + 224).
  - One fused feature DMA per batch ([128, 7840B] blob) + one upfront aux
    DMA + one output DMA = 10 DMAs total (v2: 42). DMA engines move
    8*128*7840B = 8.03MB/core ~= 22.3us at the modeled 360GB/s.
  - d2 assembly on Pool (SBUF-only), mask+multiply+accumulate stt on DVE
    reading num from PSUM, unchanged from v2.
  - Per-(batch,view,group) partial sums land in an accumulator tile; final
    reduction on host (the all-reduce of the sharding hint), along with
    exact fp32 mask counts for the denominators.
"""

import sys

sys.path.insert(0, "/opt/trn_rl_repo")

import ml_dtypes
import numpy as np

import concourse.bass as bass
import concourse.mybir as mybir
import concourse.tile as tile
from concourse import bacc
from concourse.bass import broadcast_tensor_aps
from concourse.bass_utils import run_bass_kernel_spmd

B, C, H, W = 64, 256, 28, 28
N = H * W  # 784
NCORES = 8
BPC = B // NCORES  # batches per core
G = 4  # image rows of n per tile
P = G * 28  # 112 partitions per tile
NT = N // P  # 7 tiles, exact
THR = 0.7

F32 = mybir.dt.float32
F8 = mybir.dt.float8e4
FP8_NP = ml_dtypes.float8_e4m3
ALU = mybir.AluOpType
ENG = mybir.EngineType
DOUBLE_ROW = mybir.MatmulPerfMode.DoubleRow

_COMPILED = {}


def _kgroups(WW):
    # pack as many k-tiles per PSUM bank (512 fp32) as fit: amortizes the
    # PSUM access cost of each mask+accumulate op and frees banks
    if 3 * WW * 28 <= 512:
        return [(0, 1, 2), (3, 4, 5), (6,)]
    if 2 * WW * 28 <= 512:
        return [(0, 1), (2, 3), (4, 5), (6,)]
    return [(k,) for k in range(NT)]


def _build_nc(WW=6):
    WCOL = WW * 28
    AUXW = NT * WW + 28 + 2  # dyw | dxp | thr packed per partition
    YB = 2 * N  # y bytes/partition per view (cc, n) fp8
    ZB = 2 * NT * WCOL  # z window bytes/partition per view (cc, k, w)
    VB = YB + ZB
    kgroups = _kgroups(WW)
    NG = len(kgroups)

    nc = bacc.Bacc("TRN2", debug=False, num_devices=NCORES)

    ins = {
        # per-(batch, view) fused feature blob: y-pack [128,2cc,N] then
        # z-window pack [128,2cc,NT,WCOL], all fp8 -> one DMA per view pair
        "feat": nc.dram_tensor("feat", [BPC, 2, 128, VB], F8, kind="ExternalInput"),
        # all batches' aux tables in one upfront DMA
        "aux": nc.dram_tensor("aux", [P, BPC * AUXW], F32, kind="ExternalInput"),
    }
    out = nc.dram_tensor("out", [P, BPC * 2 * NG], F32, kind="ExternalOutput")

    # Engine split (walrus: Pool supports TensorTensor but NOT TensorScalarPtr,
    # and only DVE/Pool can read two tensors while only DVE can touch PSUM):
    # d2 assembly -> Pool (SBUF-only), masked-accumulate stt -> DVE.
    with tile.TileContext(nc) as tc:
        with (
            tc.tile_pool(name="feat", bufs=3) as feat_pool,
            tc.tile_pool(name="aux", bufs=1) as aux_pool,
            tc.tile_pool(name="d2", bufs=3) as d2_pool,
            tc.tile_pool(name="acc", bufs=1) as acc_pool,
            tc.tile_pool(name="psum", bufs=1, space="PSUM") as psum_pool,
        ):
            # every (b, v, g) column is written exactly once by one engine
            ms_all = acc_pool.tile([P, BPC, 2, NG], F32, name="ms_all", tag="ms")

            aux_t = aux_pool.tile([P, BPC * AUXW], F32, tag="aux")

            for b in range(BPC):
                fts = []
                for v in range(2):
                    ftv = feat_pool.tile([128, VB], F8, tag=f"feat{v}")
                    nc.sync.dma_start(ftv[:, :], ins["feat"][b, v])
                    fts.append(ftv)
                    if b == 0 and v == 0:
                        # aux rides right after the first feature half so
                        # d2 assembly can start while view-1 data streams
                        nc.sync.dma_start(aux_t[:, :], ins["aux"][:, :])

                a0 = b * AUXW
                dyw_v = aux_t[:, a0 : a0 + NT * WW].rearrange(
                    "p (k w) -> p k w", k=NT
                )
                dxp_v = aux_t[:, a0 + NT * WW : a0 + NT * WW + 28]
                thr_v = aux_t[:, a0 + NT * WW + 28 : a0 + NT * WW + 30]

                for g, ks in enumerate(kgroups):
                    L = len(ks)
                    d2 = d2_pool.tile([P, L, WCOL], F32, tag=f"d2_{g}")
                    i0, i1 = broadcast_tensor_aps(
                        dyw_v[:, ks[0] : ks[0] + L, :, None],
                        dxp_v[:, None, None, :],
                    )
                    d2r = d2[:, :, :].rearrange("q l (a c) -> q l a c", a=WW)
                    nc.gpsimd.tensor_tensor(d2r, i0, i1, ALU.add)
                    for v in range(2):
                        yv = fts[v][:, 0:YB].rearrange("p (c n) -> p c n", c=2)
                        zv = fts[v][:, YB:VB].rearrange(
                            "p (c k w) -> p c k w", c=2, k=NT
                        )
                        num = psum_pool.tile([P, L, WCOL], F32, tag=f"num{g}_{v}")
                        for li, k in enumerate(ks):
                            # full 256-channel contraction in one DoubleRow
                            # fp8 matmul: lhsT [128,2,112], rhs [128,2,WCOL]
                            nc.tensor.matmul(
                                num[:, li, :],
                                yv[:, :, k * P : (k + 1) * P],
                                zv[:, :, k, :],
                                start=True,
                                stop=True,
                                perf_mode=DOUBLE_ROW,
                            )
                        nc.vector.scalar_tensor_tensor(
                            out=num[:, :, :],
                            in0=d2[:, :, :],
                            scalar=thr_v[:, v : v + 1],
                            in1=num[:, :, :],
                            op0=ALU.is_le,
                            op1=ALU.mult,
                            accum_out=ms_all[:, b, v, g : g + 1],
                        )

            nc.sync.dma_start(out[:, :], ms_all[:, :, :, :])

    nc.compile()
    return nc


def _get_nc(WW):
    if WW not in _COMPILED:
        _COMPILED[WW] = _build_nc(WW)
    return _COMPILED[WW]


def _prep_host(y1, y2, z1, z2, view1_grid, view2_grid):
    """Host-side prep: separable distance tables, norms, counts, shards."""
    y1f = y1.reshape(B, C, N)
    y2f = y2.reshape(B, C, N)
    z1f = z1.reshape(B, C, N)
    z2f = z2.reshape(B, C, N)

    # --- separable grid tables ------------------------------------------
    g1y = view1_grid[:, 0, :, 0]  # [B, 28]
    g1x = view1_grid[:, 1, 0, :]
    g2y = view2_grid[:, 0, :, 0]
    g2x = view2_grid[:, 1, 0, :]
    if not (
        np.array_equal(view1_grid[:, 0], np.broadcast_to(g1y[:, :, None], (B, H, W)))
        and np.array_equal(view1_grid[:, 1], np.broadcast_to(g1x[:, None, :], (B, H, W)))
        and np.array_equal(view2_grid[:, 0], np.broadcast_to(g2y[:, :, None], (B, H, W)))
        and np.array_equal(view2_grid[:, 1], np.broadcast_to(g2x[:, None, :], (B, H, W)))
    ):
        raise RuntimeError("grids are not separable; unsupported input")

    dy = g1y[:, :, None] - g2y[:, None, :]  # fp32 [B,28,28]
    dx = g1x[:, :, None] - g2x[:, None, :]
    dy2 = dy * dy
    dx2 = dx * dx

    v1bin = np.linalg.norm(view1_grid[..., 1, 1] - view1_grid[..., 0, 0], axis=-1)
    v2bin = np.linalg.norm(view2_grid[..., 1, 1] - view2_grid[..., 0, 0], axis=-1)
    t2 = np.empty((B, 2), np.float32)
    t2[:, 0] = ((THR * v1bin.astype(np.float64)) ** 2).astype(np.float32)
    t2[:, 1] = ((THR * v2bin.astype(np.float64)) ** 2).astype(np.float32)

    # --- per-(batch, tile) windows of valid i' --------------------------
    tmax2 = np.maximum(t2[:, 0], t2[:, 1]).astype(np.float64) * (1 + 1e-6)  # [B]
    first = np.zeros((B, NT), np.int64)
    width = np.zeros((B, NT), np.int64)
    anyv = np.zeros((B, NT), bool)
    for k in range(NT):
        sub_min = dy2[:, G * k : G * k + G, :].min(axis=1)  # [B, 28]
        valid = sub_min <= tmax2[:, None]  # [B, 28]
        anyv[:, k] = valid.any(axis=1)
        first[:, k] = np.argmax(valid, axis=1)
        last = 27 - np.argmax(valid[:, ::-1], axis=1)
        width[:, k] = np.where(anyv[:, k], last - first[:, k] + 1, 1)
    WW = int(max(width.max(), 4))
    if WW > 18:
        raise RuntimeError(f"mask window {WW} rows; unsupported input")
    WCOL = WW * 28

    w0 = np.minimum(np.where(anyv, first, 0), 28 - WW).astype(np.int64)  # [B, NT]

    iidx = (np.arange(P) // 28).astype(np.int64)  # [112] in 0..3
    dyw = np.zeros((B, P, NT, WW), np.float32)
    for k in range(NT):
        rows = G * k + iidx  # [112] y-side image rows
        cols = w0[:, k][:, None] + np.arange(WW)[None, :]  # [B, WW]
        dyw[:, :, k] = dy2[
            np.arange(B)[:, None, None], rows[None, :, None], cols[:, None, :]
        ]

    AUXW = NT * WW + 28 + 2
    aux = np.zeros((B, P, AUXW), np.float32)
    aux[:, :, 0 : NT * WW] = dyw.reshape(B, P, NT * WW)
    aux[:, :, NT * WW : NT * WW + 28] = np.tile(dx2, (1, G, 1))  # j = p mod 28
    aux[:, :, NT * WW + 28 : NT * WW + 30] = t2[:, None, :]

    # --- mask counts (bit-identical fp32 add + compare as device) -------
    counts = np.zeros(2, np.int64)
    for b in range(B):
        d2b = dy2[b][:, None, :, None] + dx2[b][None, :, None, :]  # fp32
        counts[0] += int((d2b <= t2[b, 0]).sum())
        counts[1] += int((d2b <= t2[b, 1]).sum())

    # --- normalized features in fp8, fused per-batch blob ---------------
    def normed8(a):
        n = np.sqrt(np.einsum("bcn,bcn->bn", a, a, dtype=np.float32))
        h = a * (1.0 / np.maximum(n, np.float32(1e-7)))[:, None, :]
        return h.reshape(B, 2, 128, N).astype(FP8_NP)  # [B, cc, part, n]

    y1h, y2h, z1h, z2h = normed8(y1f), normed8(y2f), normed8(z1f), normed8(z2f)

    # y-pack [B, v, part, cc, n]
    ypack = np.empty((B, 2, 128, 2, N), FP8_NP)
    ypack[:, 0] = y1h.transpose(0, 2, 1, 3)
    ypack[:, 1] = y2h.transpose(0, 2, 1, 3)

    # z window pack [B, v, part, cc, k, w]; v=0 pairs with z2, v=1 with z1
    zpack = np.empty((B, 2, 128, 2, NT, WCOL), FP8_NP)
    bi = np.arange(B)[:, None, None, None]  # [B,1,1,1]
    pi = np.arange(128)[None, :, None, None]  # [1,128,1,1]
    wi = w0 * 28  # [B, NT] window start columns
    cols = wi[:, :, None] + np.arange(WCOL)[None, None, :]  # [B, NT, WCOL]
    ci = cols[:, None, :, :]  # [B,1,NT,WCOL]
    for cc in range(2):
        # gather [B, part, NT, WCOL] from [B, part, N]
        zpack[:, 0, :, cc] = z2h[:, cc][bi, pi, ci]
        zpack[:, 1, :, cc] = z1h[:, cc][bi, pi, ci]

    VBB = 2 * N + 2 * NT * WCOL
    feat = np.concatenate(
        [ypack.reshape(B, 2, 128, 2 * N), zpack.reshape(B, 2, 128, 2 * NT * WCOL)],
        axis=3,
    )
    assert feat.shape == (B, 2, 128, VBB)

    in_maps = []
    for c in range(NCORES):
        s = slice(c * BPC, (c + 1) * BPC)
        in_maps.append(
            {
                "feat": feat[s],
                "aux": np.ascontiguousarray(
                    aux[s].transpose(1, 0, 2).reshape(P, BPC * AUXW)
                ),
            }
        )
    return in_maps, counts, WW


def kernel(y1, y2, z1, z2, view1_grid, view2_grid):
    y1 = np.asarray(y1, np.float32)
    y2 = np.asarray(y2, np.float32)
    z1 = np.asarray(z1, np.float32)
    z2 = np.asarray(z2, np.float32)
    view1_grid = np.asarray(view1_grid, np.float32)
    view2_grid = np.asarray(view2_grid, np.float32)

    in_maps, counts, WW = _prep_host(y1, y2, z1, z2, view1_grid, view2_grid)
    nc = _get_nc(WW)
    res = run_bass_kernel_spmd(nc, in_maps, core_ids=list(range(NCORES)))
    s = np.zeros(2, np.float64)
    for i in range(NCORES):
        o = res.results[i]["out"].astype(np.float64)  # [P, BPC*2*NG]
        ng = o.shape[1] // (BPC * 2)
        o = o.reshape(P, BPC, 2, ng)
        s += o.sum(axis=(0, 1, 3))
    loss = -(
        np.float32(s[0]) / np.float32(counts[0])
        + np.float32(s[1]) / np.float32(counts[1])
    )
    return np.array(loss, dtype=np.float32)


# revision 10
# speedup vs baseline: 1.3376x; 1.0292x over previous
"""ConsistencyLoss kernel v3 for 8 Trainium2 NeuronCores.

Math (per reference):
  For view1: sim = cos_sim_pairwise(y1, z2) [B,N,N]; mask from grid distances;
  loss_v = sum(sim*mask)/sum(mask); out = -(loss_1 + loss_2), N = 28*28 = 784.

v3 strategy (data-parallel over batch, 8 batches/core), changes vs v2:
  - Features in fp8 e4m3 (ml_dtypes.float8_e4m3 == mybir float8e4). Measured
    end-to-end rel err 5.1e-3 on the harness inputs (gate 2e-2). Halves HBM
    traffic vs fp16 AND enables the PE DoubleRow perf mode.
  - Moving (z-side) windows gathered on the HOST into a [128,2,2,NT,WCOL]
    blob: every matmul AP is compile-time static. This kills the dynamic-AP
    ISA patches (21.5us of PE.SEQ in v2) and the TensorLoads, at the cost of
    shipping z windows 1.5x (4704B vs 3136B per partition per batch).
  - DoubleRow fp8 matmul: lhsT [128,2cc,112], rhs [128,2cc,168] -> one
    matmul per (batch, view, k-tile) covers the full 256-channel contraction
    at 0.5 cycles/row. 112 matmuls + 112 ldweights total (v2: 224 + 224).
  - One fused feature DMA per batch ([128, 7840B] blob) + one upfront aux
    DMA + one output DMA = 10 DMAs total (v2: 42). DMA engines move
    8*128*7840B = 8.03MB/core ~= 22.3us at the modeled 360GB/s.
  - d2 assembly on Pool (SBUF-only), mask+multiply+accumulate stt on DVE
    reading num from PSUM, unchanged from v2.
  - Per-(batch,view,group) partial sums land in an accumulator tile; final
    reduction on host (the all-reduce of the sharding hint), along with
    exact fp32 mask counts for the denominators.
"""

import sys

sys.path.insert(0, "/opt/trn_rl_repo")

import ml_dtypes
import numpy as np

import concourse.bass as bass
import concourse.mybir as mybir
import concourse.tile as tile
from concourse import bacc
from concourse.bass import broadcast_tensor_aps
from concourse.bass_utils import run_bass_kernel_spmd

B, C, H, W = 64, 256, 28, 28
N = H * W  # 784
NCORES = 8
BPC = B // NCORES  # batches per core
G = 4  # image rows of n per tile
P = G * 28  # 112 partitions per tile
NT = N // P  # 7 tiles, exact
THR = 0.7

F32 = mybir.dt.float32
F8 = mybir.dt.float8e4
FP8_NP = ml_dtypes.float8_e4m3
ALU = mybir.AluOpType
ENG = mybir.EngineType
DOUBLE_ROW = mybir.MatmulPerfMode.DoubleRow

_COMPILED = {}


def _kgroups(WW):
    # pack as many k-tiles per PSUM bank (512 fp32) as fit: amortizes the
    # PSUM access cost of each mask+accumulate op and frees banks
    if 3 * WW * 28 <= 512:
        return [(0, 1, 2), (3, 4, 5), (6,)]
    if 2 * WW * 28 <= 512:
        return [(0, 1), (2, 3), (4, 5), (6,)]
    return [(k,) for k in range(NT)]


def _build_nc(WW=6):
    WCOL = WW * 28
    AUXW = NT * WW + 28 + 2  # dyw | dxp | thr packed per partition
    YB = 2 * N  # y bytes/partition per view (cc, n) fp8
    ZB = 2 * NT * WCOL  # z window bytes/partition per view (cc, k, w)
    VB = YB + ZB
    kgroups = _kgroups(WW)
    NG = len(kgroups)

    nc = bacc.Bacc("TRN2", debug=False, num_devices=NCORES)

    ins = {
        # per-(batch, view) fused feature blob: y-pack [128,2cc,N] then
        # z-window pack [128,2cc,NT,WCOL], all fp8 -> one DMA per view pair
        "feat": nc.dram_tensor("feat", [BPC, 2, 128, VB], F8, kind="ExternalInput"),
        # all batches' aux tables in one upfront DMA
        "aux": nc.dram_tensor("aux", [P, BPC * AUXW], F32, kind="ExternalInput"),
    }
    out = nc.dram_tensor("out", [P, BPC * 2 * NG], F32, kind="ExternalOutput")

    # Engine split (walrus: Pool supports TensorTensor but NOT TensorScalarPtr,
    # and only DVE/Pool can read two tensors while only DVE can touch PSUM):
    # d2 assembly -> Pool (SBUF-only), masked-accumulate stt -> DVE.
    with tile.TileContext(nc) as tc:
        with (
            tc.tile_pool(name="feat", bufs=3) as feat_pool,
            tc.tile_pool(name="aux", bufs=1) as aux_pool,
            tc.tile_pool(name="d2", bufs=3) as d2_pool,
            tc.tile_pool(name="acc", bufs=1) as acc_pool,
            tc.tile_pool(name="psum", bufs=1, space="PSUM") as psum_pool,
        ):
            # every (b, v, g) column is written exactly once by one engine
            ms_all = acc_pool.tile([P, BPC, 2, NG], F32, name="ms_all", tag="ms")

            aux_t = aux_pool.tile([P, BPC * AUXW], F32, tag="aux")
            # batch-0's aux slice ships first (tiny) so Pool d2 starts ASAP;
            # the rest follows the first two feature halves
            nc.sync.dma_start(aux_t[:, 0:AUXW], ins["aux"][:, 0:AUXW])

            for b in range(BPC):
                fts = []
                for v in range(2):
                    ftv = feat_pool.tile([128, VB], F8, tag=f"feat{v}")
                    nc.sync.dma_start(ftv[:, :], ins["feat"][b, v])
                    fts.append(ftv)
                if b == 0:
                    nc.sync.dma_start(
                        aux_t[:, AUXW:], ins["aux"][:, AUXW:]
                    )

                a0 = b * AUXW
                dyw_v = aux_t[:, a0 : a0 + NT * WW].rearrange(
                    "p (k w) -> p k w", k=NT
                )
                dxp_v = aux_t[:, a0 + NT * WW : a0 + NT * WW + 28]
                thr_v = aux_t[:, a0 + NT * WW + 28 : a0 + NT * WW + 30]

                d2s = []
                for g, ks in enumerate(kgroups):
                    L = len(ks)
                    d2 = d2_pool.tile([P, L, WCOL], F32, tag=f"d2_{g}")
                    i0, i1 = broadcast_tensor_aps(
                        dyw_v[:, ks[0] : ks[0] + L, :, None],
                        dxp_v[:, None, None, :],
                    )
                    d2r = d2[:, :, :].rearrange("q l (a c) -> q l a c", a=WW)
                    nc.gpsimd.tensor_tensor(d2r, i0, i1, ALU.add)
                    d2s.append(d2)

                # view-outer order: each feature half feeds NG consecutive
                # stt ops, so the DVE stream never waits on the other half
                for v in range(2):
                    yv = fts[v][:, 0:YB].rearrange("p (c n) -> p c n", c=2)
                    zv = fts[v][:, YB:VB].rearrange(
                        "p (c k w) -> p c k w", c=2, k=NT
                    )
                    for g, ks in enumerate(kgroups):
                        L = len(ks)
                        num = psum_pool.tile([P, L, WCOL], F32, tag=f"num{g}_{v}")
                        for li, k in enumerate(ks):
                            # full 256-channel contraction in one DoubleRow
                            # fp8 matmul: lhsT [128,2,112], rhs [128,2,WCOL]
                            nc.tensor.matmul(
                                num[:, li, :],
                                yv[:, :, k * P : (k + 1) * P],
                                zv[:, :, k, :],
                                start=True,
                                stop=True,
                                perf_mode=DOUBLE_ROW,
                            )
                        nc.vector.scalar_tensor_tensor(
                            out=num[:, :, :],
                            in0=d2s[g][:, :, :],
                            scalar=thr_v[:, v : v + 1],
                            in1=num[:, :, :],
                            op0=ALU.is_le,
                            op1=ALU.mult,
                            accum_out=ms_all[:, b, v, g : g + 1],
                        )

                if b == BPC - 2:
                    # drain all-but-last batches' sums early; the final DMA
                    # then only waits on the last batch's six stt columns
                    nc.sync.dma_start(
                        out[:, 0 : (BPC - 1) * 2 * NG],
                        ms_all[:, 0 : BPC - 1],
                    )
            nc.sync.dma_start(
                out[:, (BPC - 1) * 2 * NG :], ms_all[:, BPC - 1 :]
            )

    nc.compile()
    return nc


def _get_nc(WW):
    if WW not in _COMPILED:
        _COMPILED[WW] = _build_nc(WW)
    return _COMPILED[WW]


def _prep_host(y1, y2, z1, z2, view1_grid, view2_grid):
    """Host-side prep: separable distance tables, norms, counts, shards."""
    y1f = y1.reshape(B, C, N)
    y2f = y2.reshape(B, C, N)
    z1f = z1.reshape(B, C, N)
    z2f = z2.reshape(B, C, N)

    # --- separable grid tables ------------------------------------------
    g1y = view1_grid[:, 0, :, 0]  # [B, 28]
    g1x = view1_grid[:, 1, 0, :]
    g2y = view2_grid[:, 0, :, 0]
    g2x = view2_grid[:, 1, 0, :]
    if not (
        np.array_equal(view1_grid[:, 0], np.broadcast_to(g1y[:, :, None], (B, H, W)))
        and np.array_equal(view1_grid[:, 1], np.broadcast_to(g1x[:, None, :], (B, H, W)))
        and np.array_equal(view2_grid[:, 0], np.broadcast_to(g2y[:, :, None], (B, H, W)))
        and np.array_equal(view2_grid[:, 1], np.broadcast_to(g2x[:, None, :], (B, H, W)))
    ):
        raise RuntimeError("grids are not separable; unsupported input")

    dy = g1y[:, :, None] - g2y[:, None, :]  # fp32 [B,28,28]
    dx = g1x[:, :, None] - g2x[:, None, :]
    dy2 = dy * dy
    dx2 = dx * dx

    v1bin = np.linalg.norm(view1_grid[..., 1, 1] - view1_grid[..., 0, 0], axis=-1)
    v2bin = np.linalg.norm(view2_grid[..., 1, 1] - view2_grid[..., 0, 0], axis=-1)
    t2 = np.empty((B, 2), np.float32)
    t2[:, 0] = ((THR * v1bin.astype(np.float64)) ** 2).astype(np.float32)
    t2[:, 1] = ((THR * v2bin.astype(np.float64)) ** 2).astype(np.float32)

    # --- per-(batch, tile) windows of valid i' --------------------------
    tmax2 = np.maximum(t2[:, 0], t2[:, 1]).astype(np.float64) * (1 + 1e-6)  # [B]
    first = np.zeros((B, NT), np.int64)
    width = np.zeros((B, NT), np.int64)
    anyv = np.zeros((B, NT), bool)
    for k in range(NT):
        sub_min = dy2[:, G * k : G * k + G, :].min(axis=1)  # [B, 28]
        valid = sub_min <= tmax2[:, None]  # [B, 28]
        anyv[:, k] = valid.any(axis=1)
        first[:, k] = np.argmax(valid, axis=1)
        last = 27 - np.argmax(valid[:, ::-1], axis=1)
        width[:, k] = np.where(anyv[:, k], last - first[:, k] + 1, 1)
    WW = int(max(width.max(), 4))
    if WW > 18:
        raise RuntimeError(f"mask window {WW} rows; unsupported input")
    WCOL = WW * 28

    w0 = np.minimum(np.where(anyv, first, 0), 28 - WW).astype(np.int64)  # [B, NT]

    iidx = (np.arange(P) // 28).astype(np.int64)  # [112] in 0..3
    dyw = np.zeros((B, P, NT, WW), np.float32)
    for k in range(NT):
        rows = G * k + iidx  # [112] y-side image rows
        cols = w0[:, k][:, None] + np.arange(WW)[None, :]  # [B, WW]
        dyw[:, :, k] = dy2[
            np.arange(B)[:, None, None], rows[None, :, None], cols[:, None, :]
        ]

    AUXW = NT * WW + 28 + 2
    aux = np.zeros((B, P, AUXW), np.float32)
    aux[:, :, 0 : NT * WW] = dyw.reshape(B, P, NT * WW)
    aux[:, :, NT * WW : NT * WW + 28] = np.tile(dx2, (1, G, 1))  # j = p mod 28
    aux[:, :, NT * WW + 28 : NT * WW + 30] = t2[:, None, :]

    # --- mask counts (bit-identical fp32 add + compare as device) -------
    counts = np.zeros(2, np.int64)
    for b in range(B):
        d2b = dy2[b][:, None, :, None] + dx2[b][None, :, None, :]  # fp32
        counts[0] += int((d2b <= t2[b, 0]).sum())
        counts[1] += int((d2b <= t2[b, 1]).sum())

    # --- normalized features in fp8, fused per-batch blob ---------------
    def normed8(a):
        n = np.sqrt(np.einsum("bcn,bcn->bn", a, a, dtype=np.float32))
        h = a * (1.0 / np.maximum(n, np.float32(1e-7)))[:, None, :]
        return h.reshape(B, 2, 128, N).astype(FP8_NP)  # [B, cc, part, n]

    y1h, y2h, z1h, z2h = normed8(y1f), normed8(y2f), normed8(z1f), normed8(z2f)

    # y-pack [B, v, part, cc, n]
    ypack = np.empty((B, 2, 128, 2, N), FP8_NP)
    ypack[:, 0] = y1h.transpose(0, 2, 1, 3)
    ypack[:, 1] = y2h.transpose(0, 2, 1, 3)

    # z window pack [B, v, part, cc, k, w]; v=0 pairs with z2, v=1 with z1
    zpack = np.empty((B, 2, 128, 2, NT, WCOL), FP8_NP)
    bi = np.arange(B)[:, None, None, None]  # [B,1,1,1]
    pi = np.arange(128)[None, :, None, None]  # [1,128,1,1]
    wi = w0 * 28  # [B, NT] window start columns
    cols = wi[:, :, None] + np.arange(WCOL)[None, None, :]  # [B, NT, WCOL]
    ci = cols[:, None, :, :]  # [B,1,NT,WCOL]
    for cc in range(2):
        # gather [B, part, NT, WCOL] from [B, part, N]
        zpack[:, 0, :, cc] = z2h[:, cc][bi, pi, ci]
        zpack[:, 1, :, cc] = z1h[:, cc][bi, pi, ci]

    VBB = 2 * N + 2 * NT * WCOL
    feat = np.concatenate(
        [ypack.reshape(B, 2, 128, 2 * N), zpack.reshape(B, 2, 128, 2 * NT * WCOL)],
        axis=3,
    )
    assert feat.shape == (B, 2, 128, VBB)

    in_maps = []
    for c in range(NCORES):
        s = slice(c * BPC, (c + 1) * BPC)
        in_maps.append(
            {
                "feat": feat[s],
                "aux": np.ascontiguousarray(
                    aux[s].transpose(1, 0, 2).reshape(P, BPC * AUXW)
                ),
            }
        )
    return in_maps, counts, WW


def kernel(y1, y2, z1, z2, view1_grid, view2_grid):
    y1 = np.asarray(y1, np.float32)
    y2 = np.asarray(y2, np.float32)
    z1 = np.asarray(z1, np.float32)
    z2 = np.asarray(z2, np.float32)
    view1_grid = np.asarray(view1_grid, np.float32)
    view2_grid = np.asarray(view2_grid, np.float32)

    in_maps, counts, WW = _prep_host(y1, y2, z1, z2, view1_grid, view2_grid)
    nc = _get_nc(WW)
    res = run_bass_kernel_spmd(nc, in_maps, core_ids=list(range(NCORES)))
    s = np.zeros(2, np.float64)
    for i in range(NCORES):
        o = res.results[i]["out"].astype(np.float64)  # [P, BPC*2*NG]
        ng = o.shape[1] // (BPC * 2)
        o = o.reshape(P, BPC, 2, ng)
        s += o.sum(axis=(0, 1, 3))
    loss = -(
        np.float32(s[0]) / np.float32(counts[0])
        + np.float32(s[1]) / np.float32(counts[1])
    )
    return np.array(loss, dtype=np.float32)
